# revision 1
# baseline (speedup 1.0000x reference)
"""GCN2 ensemble kernel for Trainium2, 8-way node-sharded.

Design:
  - Nodes padded to N_pad = n_cores * SHARD, SHARD = TPC*128 rows per core.
  - Edges partitioned by dst across cores; within a core grouped by
    (dst tile, src bucket); each group padded to a multiple of 128 "chunk"
    edges (pad idx=-1 -> dma_gather skips, weight=0 -> no contribution).
  - Chunk schedule (chunks per (tile,bucket)) is the max over cores so the
    single SPMD program fits every core's data.
  - SpMM: dma_gather 768B bf16 rows of h (3 nets fused, F=384) into
    G[128 edges, nch, 384]; S[128e,128dst] one-hot built on DVE from
    iota==dst_local times edge weight; PE matmul agg += S.T @ G per chunk.
  - Dense: t = agg + 0.1*x0 ; u = t @ (beta/(1-beta) cw) via PE transpose;
    h' = relu((1-beta)*(t+u)) on ACT.
  - AllGather of h shard -> shared h_full per layer (parity double-buffered).
"""
import dataclasses
import math
import numpy as np
import ml_dtypes

import os as _os
import concourse.bacc as bacc
import concourse.bass as bass
import concourse.mybir as mybir
import concourse.tile as tile

BF16 = np.dtype(np.float16)
P = 128
ALPHA = 0.1
THETA = 0.5


class Cfg:
    def __init__(self, N=100000, IN=128, H=128, OUT=112, L=4, NETS=3,
                 n_cores=8, n_buckets=4, t_sup=4, sg=16, reps=1, ag_shared=True):
        self.N, self.IN, self.H, self.OUT, self.L, self.NETS = N, IN, H, OUT, L, NETS
        self.n_cores = n_cores
        self.F = NETS * H
        self.TPC = math.ceil(N / (n_cores * P))       # tiles per core
        self.SHARD = self.TPC * P
        self.N_pad = n_cores * self.SHARD
        self.NB = n_buckets
        assert self.N_pad % n_buckets == 0
        self.BUCKET = self.N_pad // n_buckets
        assert self.BUCKET <= 32768
        self.T_SUP = t_sup
        self.SG = sg                                   # chunks per S-build group
        self.n_sup = math.ceil(self.TPC / t_sup)
        self.reps = reps
        self.ag_shared = ag_shared
        self.betas = [float(np.log(THETA / (l + 1) + 1.0)) for l in range(L)]


def host_prep(cfg, x, edge_index, edge_weight,
              lin0_w, lin0_b, lin1_w, lin1_b, conv_w):
    """Build per-core input maps + the static chunk schedule."""
    c = cfg
    src = np.asarray(edge_index[0], dtype=np.int64)
    dst = np.asarray(edge_index[1], dtype=np.int64)
    w = np.asarray(edge_weight, dtype=np.float32)

    core = dst // c.SHARD
    tloc = (dst % c.SHARD) // P
    dloc = (dst % P).astype(np.int32)
    buck = src // c.BUCKET
    sloc = (src % c.BUCKET).astype(np.int32)

    # counts per (core, tile, bucket)
    cnt = np.zeros((c.n_cores, c.TPC, c.NB), np.int64)
    np.add.at(cnt, (core, tloc, buck), 1)
    nch = np.ceil(cnt / P).astype(np.int64).max(axis=0)  # [TPC, NB]
    # real tiles with zero chunks still need one dummy chunk so PSUM is written
    first_real_rows = np.arange(c.TPC) * P  # row offset within shard
    for t in range(c.TPC):
        if nch[t].sum() == 0:
            # tile is all-pad iff its first global row >= N on EVERY core;
            # core 0's row for tile t is t*128 which is < N always, so only
            # high cores have pad tiles. We need per-core knowledge: a tile is
            # "real" for core k iff k*SHARD + t*128 < N. The schedule is
            # shared, so give the tile a dummy chunk if real for ANY core.
            if any(k * c.SHARD + t * P < c.N for k in range(c.n_cores)):
                nch[t][0] = 1

    # schedule: for sup, for b, for t in sup: nch[t][b] chunks
    sup_tiles = [list(range(s * c.T_SUP, min((s + 1) * c.T_SUP, c.TPC)))
                 for s in range(c.n_sup)]
    chunk_tile = []        # global chunk idx -> tile (core-local)
    call_list = []         # (sup, b, nidx, idx_col_off, chunk0)
    idx_col_off = 0
    for s, tiles in enumerate(sup_tiles):
        for b in range(c.NB):
            ncall = int(sum(nch[t][b] for t in tiles))
            if ncall == 0:
                continue
            call_list.append((s, b, ncall * P, idx_col_off, len(chunk_tile)))
            idx_col_off += ncall * P // 16
            for t in tiles:
                chunk_tile.extend([t] * int(nch[t][b]))
    tot_chunks = len(chunk_tile)
    idx_cols = idx_col_off

    # start/stop flags per chunk (per tile, over the whole schedule)
    first_chunk = {}
    last_chunk = {}
    for ci, t in enumerate(chunk_tile):
        first_chunk.setdefault(t, ci)
        last_chunk[t] = ci
    start_flag = [first_chunk[t] == ci for ci, t in enumerate(chunk_tile)]
    stop_flag = [last_chunk[t] == ci for ci, t in enumerate(chunk_tile)]

    # chunk idx-position base: chunk ci occupies idx positions within its call
    # Build per-core edge placement
    # group offset within schedule: for (t,b) -> (call chunk0 within call, ...)
    # compute for each (s,b,t): chunk base (global) in schedule order
    group_chunk0 = {}
    ci = 0
    for s, tiles in enumerate(sup_tiles):
        for b in range(c.NB):
            if sum(nch[t][b] for t in tiles) == 0:
                continue
            for t in tiles:
                if nch[t][b] > 0:
                    group_chunk0[(t, b)] = ci
                    ci += int(nch[t][b])
    assert ci == tot_chunks

    # map global chunk -> call, and position-in-call
    chunk_call = np.zeros(tot_chunks, np.int64)
    chunk_pos_in_call = np.zeros(tot_chunks, np.int64)
    for k, (s, b, nidx, coloff, ch0) in enumerate(call_list):
        nchk = nidx // P
        chunk_call[ch0:ch0 + nchk] = k
        chunk_pos_in_call[ch0:ch0 + nchk] = np.arange(nchk)

    # per-core arrays
    order = np.lexsort((sloc, buck, tloc, core))  # sort edges
    src_s, w_s = sloc[order], w[order]
    core_s, tloc_s, buck_s, dloc_s = core[order], tloc[order], buck[order], dloc[order]

    schedule = dict(call_list=call_list, chunk_tile=chunk_tile,
                    start_flag=start_flag, stop_flag=stop_flag,
                    tot_chunks=tot_chunks, idx_cols=idx_cols,
                    sup_tiles=sup_tiles, nch=nch)

    in_maps = []
    # weights (shared across cores)
    w0cat = np.ascontiguousarray(
        np.concatenate([lin0_w[n] for n in range(c.NETS)], axis=1)).astype(BF16)
    b0rep = np.tile(np.concatenate([lin0_b[n] for n in range(c.NETS)])[None, :],
                    (P, 1)).astype(np.float32)
    w1cat = np.ascontiguousarray(
        np.concatenate([lin1_w[n] for n in range(c.NETS)], axis=1)).astype(BF16)
    b1rep = np.tile(np.concatenate([lin1_b[n] for n in range(c.NETS)])[None, :],
                    (P, 1)).astype(np.float32)
    cwcat = np.zeros((c.L, c.H, c.F), np.float32)
    for l in range(c.L):
        beta = c.betas[l]
        for n in range(c.NETS):
            cwcat[l][:, n * c.H:(n + 1) * c.H] = conv_w[n, l] * (beta / (1.0 - beta))
    cwcat = cwcat.astype(BF16)
    iota = np.tile(np.arange(P, dtype=np.float32)[None, :], (P, 1)).astype(BF16)
    ident_b = np.eye(P, dtype=np.float32).astype(BF16)
    ident_f = np.eye(P, dtype=np.float32)

    edge_pos = np.zeros(len(src_s), np.int64)  # idx position in schedule-space
    # compute per-core per-group placement via sorted runs
    for k in range(c.n_cores):
        msk = core_s == k
        # idx array in "global idx position" space: chunk ci, slot p -> ci*128+p
        idx_flat = np.zeros(tot_chunks * P, np.int32)
        dst_flat = np.zeros(tot_chunks * P, np.int32)
        wv_flat = np.zeros(tot_chunks * P, np.float32)
        tl, bk = tloc_s[msk], buck_s[msk]
        gkey = tl * c.NB + bk
        # stable order within (t,b) is by sloc already
        uniq, starts_counts = np.unique(gkey, return_index=False, return_counts=True), None
        gkeys, counts = np.unique(gkey, return_counts=True)
        offs = np.zeros_like(counts)
        pos_in_group = np.arange(len(gkey)) - np.repeat(
            np.concatenate([[0], np.cumsum(counts)[:-1]]), counts)
        group_base = np.array([group_chunk0[(gk // c.NB, gk % c.NB)] * P
                               for gk in gkeys])
        base_per_edge = np.repeat(group_base, counts)
        pos = base_per_edge + pos_in_group
        idx_flat[pos] = src_s[msk]
        dst_flat[pos] = dloc_s[msk]
        wv_flat[pos] = w_s[msk] * (1.0 - ALPHA)

        # wrap idx per call into [128, idx_cols] int16
        idx_all = np.zeros((P, idx_cols), np.int16)
        for (s, b, nidx, coloff, ch0) in call_list:
            a = idx_flat[ch0 * P: ch0 * P + nidx]
            wrapped = a.reshape(nidx // 16, 16).T.astype(np.int16)  # [16, nidx/16]
            idx_all[:, coloff:coloff + nidx // 16] = np.tile(wrapped, (8, 1))
        # precomputed one-hot*weight S: sm[p, ci*128 + d] = w_edge(ci*128+p)
        # iff d == dst_local(edge), else 0. Used directly as matmul lhsT.
        allpos = np.arange(tot_chunks * P)
        sm_all = np.zeros((P, tot_chunks * P), BF16)
        sm_all[allpos % P, (allpos // P) * P + dst_flat] = wv_flat.astype(BF16)

        xs = np.zeros((c.SHARD, c.IN), np.float32)
        lo, hi = k * c.SHARD, min((k + 1) * c.SHARD, c.N)
        if hi > lo:
            xs[:hi - lo] = np.asarray(x[lo:hi], np.float32)

        in_maps.append({
            "xs": xs, "idx_all": idx_all, "sm_all": sm_all,
            "w0cat": w0cat, "b0rep": b0rep, "w1cat": w1cat, "b1rep": b1rep,
            "cwcat": cwcat, "ident_b": ident_b, "ident_f": ident_f,
        })
    return in_maps, schedule


def build_nc(cfg, schedule):
    c = cfg
    ABL = set(_os.environ.get("GCN_ABLATE", "").split(","))
    nch = schedule["nch"]
    call_list = schedule["call_list"]
    chunk_tile = schedule["chunk_tile"]
    start_flag = schedule["start_flag"]
    stop_flag = schedule["stop_flag"]
    sup_tiles = schedule["sup_tiles"]
    idx_cols = schedule["idx_cols"]
    tot_chunks = schedule["tot_chunks"]
    FP32 = mybir.dt.float32
    BF = mybir.dt.float16

    nc = bacc.Bacc("TRN2", target_bir_lowering=False, debug=False,
                   num_devices=c.n_cores, num_swdge_queues=4)
    # I/O
    xs = nc.dram_tensor("xs", [c.SHARD, c.IN], FP32, kind="ExternalInput").ap()
    idx_all_d = nc.dram_tensor("idx_all", [P, idx_cols], mybir.dt.int16,
                               kind="ExternalInput").ap()
    sm_d = nc.dram_tensor("sm_all", [P, tot_chunks * P], BF,
                          kind="ExternalInput").ap()
    w0cat_d = nc.dram_tensor("w0cat", [c.IN, c.F], BF, kind="ExternalInput").ap()
    b0rep_d = nc.dram_tensor("b0rep", [P, c.F], FP32, kind="ExternalInput").ap()
    w1cat_d = nc.dram_tensor("w1cat", [c.H, c.NETS * c.OUT], BF,
                             kind="ExternalInput").ap()
    b1rep_d = nc.dram_tensor("b1rep", [P, c.NETS * c.OUT], FP32,
                             kind="ExternalInput").ap()
    cwcat_d = nc.dram_tensor("cwcat", [c.L, c.H, c.F], BF, kind="ExternalInput").ap()
    identb_d = nc.dram_tensor("ident_b", [P, P], BF, kind="ExternalInput").ap()
    identf_d = nc.dram_tensor("ident_f", [P, P], FP32, kind="ExternalInput").ap()
    out_d = nc.dram_tensor("out", [c.SHARD, c.OUT], FP32, kind="ExternalOutput").ap()

    n_real_tiles = [t for t in range(c.TPC)]  # compute-skip handled per tile below

    with tile.TileContext(nc) as tc:
        with tc.tile_pool(name="dram", bufs=1, space="DRAM") as dram, \
             tc.tile_pool(name="const", bufs=1) as cst:
            h_shard = [dram.tile([c.SHARD, c.F], BF, name=f"h_shard{p}")
                       for p in range(2)]
            h_full_space = "Shared" if (c.n_cores > 4 and c.ag_shared) else "Local"
            # Shared DRAM allows only a single writer instruction: one tensor
            # per AllGather instance (rep, layer).
            h_full_all = [[dram.tile([c.N_pad, c.F], BF, addr_space=h_full_space,
                                     name=f"h_full_r{r}_l{l}")
                           for l in range(c.L)] for r in range(c.reps)]
            x0s_hbm = dram.tile([c.SHARD, c.F], BF, name="x0s_hbm")
            h4_hbm = dram.tile([c.SHARD, c.F], BF, name="h4_hbm")

            # resident constants
            w0_sb = cst.tile([P, c.F], BF)
            b0_sb = cst.tile([P, c.F], FP32)
            w1_sb = cst.tile([P, c.NETS * c.OUT], BF)
            b1_sb = cst.tile([P, c.NETS * c.OUT], FP32)
            cw_sb = cst.tile([P, c.L * c.F], BF)
            idb_sb = cst.tile([P, P], BF)
            idf_sb = cst.tile([P, P], FP32)
            nc.sync.dma_start(out=w0_sb[:], in_=w0cat_d)
            nc.sync.dma_start(out=b0_sb[:], in_=b0rep_d)
            nc.sync.dma_start(out=w1_sb[:], in_=w1cat_d)
            nc.sync.dma_start(out=b1_sb[:], in_=b1rep_d)
            for l in range(c.L):
                nc.sync.dma_start(out=cw_sb[:, l * c.F:(l + 1) * c.F], in_=cwcat_d[l])
            nc.sync.dma_start(out=idb_sb[:], in_=identb_d)
            nc.sync.dma_start(out=idf_sb[:], in_=identf_d)

            def tile_is_real(t):
                # real on at least core 0 (rows k*SHARD + t*128); host zero-pads
                # xs so computing garbage-free everywhere; skip only if pad on
                # ALL cores (i.e. t*128 >= N - (n_cores-1)*SHARD handled via N)
                return any(k * c.SHARD + t * P < c.N for k in range(c.n_cores))

            real_tiles = [t for t in range(c.TPC) if tile_is_real(t)]

            serial_reps = "serial" in ABL
            for rep in range(c.reps):
                # ---------------- Stage A: input layer ----------------
                with tc.tile_pool(name="sA", bufs=4) as sA, \
                     tc.tile_pool(name="pA", bufs=2, space="PSUM") as pA:
                    gate = None
                    if serial_reps and rep > 0:
                        od = sA.tile([P, c.OUT], FP32, tag="od", name=f"od{rep}")
                        nc.sync.dma_start(out=od[:], in_=out_d[0:P, :])
                        g0 = sA.tile([P, 1], FP32, tag="g0", name=f"g0{rep}")
                        nc.vector.tensor_reduce(
                            out=g0[:], in_=od[:], axis=mybir.AxisListType.X,
                            op=mybir.AluOpType.max)
                        gate = sA.tile([P, 1], FP32, tag="g1", name=f"g1{rep}")
                        nc.vector.tensor_scalar(
                            out=gate[:], in0=g0[:], scalar1=0.0, scalar2=1.0,
                            op0=mybir.AluOpType.mult, op1=mybir.AluOpType.add)
                    for t in real_tiles:
                        xt = sA.tile([P, c.IN], FP32, tag="xt")
                        nc.sync.dma_start(out=xt[:], in_=xs[t * P:(t + 1) * P, :])
                        if gate is not None:
                            xg = sA.tile([P, c.IN], FP32, tag="xg")
                            nc.vector.tensor_tensor(
                                out=xg[:], in0=xt[:],
                                in1=gate[:, 0:1].to_broadcast([P, c.IN]),
                                op=mybir.AluOpType.mult)
                            xt = xg
                        xT_ps = pA.tile([P, P], FP32, tag="xT")
                        nc.tensor.transpose(out=xT_ps[:], in_=xt[:], identity=idf_sb[:])
                        xT_sb = sA.tile([P, P], BF, tag="xTs")
                        nc.scalar.activation(out=xT_sb[:], in_=xT_ps[:],
                                             func=mybir.ActivationFunctionType.Copy)
                        h0_ps = pA.tile([P, c.F], FP32, tag="h0")
                        nc.tensor.matmul(out=h0_ps[:], lhsT=xT_sb[:], rhs=w0_sb[:],
                                         start=True, stop=True)
                        hb = sA.tile([P, c.F], FP32, tag="hb")
                        nc.vector.tensor_tensor(out=hb[:], in0=h0_ps[:], in1=b0_sb[:],
                                                op=mybir.AluOpType.add)
                        h0t = sA.tile([P, c.F], BF, tag="h0t")
                        nc.scalar.activation(out=h0t[:], in_=hb[:],
                                             func=mybir.ActivationFunctionType.Relu)
                        x0t = sA.tile([P, c.F], BF, tag="x0t")
                        nc.vector.tensor_scalar(out=x0t[:], in0=h0t[:],
                                                scalar1=ALPHA, scalar2=None,
                                                op0=mybir.AluOpType.mult)
                        nc.sync.dma_start(
                            out=h_shard[0][t * P:(t + 1) * P, :], in_=h0t[:])
                        nc.sync.dma_start(
                            out=x0s_hbm[t * P:(t + 1) * P, :], in_=x0t[:])

                # AG0
                h_full = h_full_all[rep]
                if "noag" not in ABL:
                    nc.gpsimd.collective_compute(
                        "AllGather", mybir.AluOpType.bypass,
                        replica_groups=[list(range(c.n_cores))],
                        ins=[h_shard[0][:, :]], outs=[h_full[0][:, :]])
                else:
                    nc.sync.dma_start(
                        out=h_full[0][:c.SHARD, :], in_=h_shard[0][:, :])

                # ---------------- Stage B: GCN2 layers ----------------
                with tc.tile_pool(name="gpool", bufs=2) as gpool, \
                     tc.tile_pool(name="spool", bufs=2) as spool, \
                     tc.tile_pool(name="tpool", bufs=6) as tpool, \
                     tc.tile_pool(name="psA", bufs=4, space="PSUM") as ps_agg, \
                     tc.tile_pool(name="psT", bufs=2, space="PSUM") as ps_t, \
                     tc.tile_pool(name="psU", bufs=2, space="PSUM") as ps_u:
                    for l in range(c.L):
                        pr, pw = l % 2, (l + 1) % 2
                        beta = c.betas[l]
                        gather_cnt = 0
                        for s, tiles in enumerate(sup_tiles):
                            calls = [cl for cl in call_list if cl[0] == s]
                            if not calls:
                                continue
                            g_tiles = {}
                            col_lo = min(cl[3] for cl in calls)
                            col_hi = max(cl[3] + cl[2] // 16 for cl in calls)
                            ix = spool.tile([P, col_hi - col_lo],
                                            mybir.dt.int16, tag="ix",
                                            name=f"ix_{l}_{s}")
                            nc.sync.dma_start(
                                out=ix[:], in_=idx_all_d[:, col_lo:col_hi])
                            for (s_, b, nidx, coloff, ch0) in calls:
                                nchk = nidx // P
                                g = gpool.tile([P, nchk, c.F], BF, tag=f"g{b}")
                                if "nogather" in ABL:
                                    nc.vector.memset(g[:], 0)
                                if "nogather" not in ABL:
                                    nc.gpsimd.dma_gather(
                                        g[:],
                                        h_full[l][b * c.BUCKET:(b + 1) * c.BUCKET, :],
                                        ix[:, coloff - col_lo:
                                           coloff - col_lo + nidx // 16],
                                        nidx, nidx, c.F,
                                        single_packet=False,
                                        queue_num=gather_cnt % 4)
                                gather_cnt += 1
                                g_tiles[b] = (g, ch0, nchk)
                            # stream precomputed S (one-hot*weight) for this
                            # supertile's chunk range from DRAM
                            ch_lo = min(ch0 for (_, _, _, _, ch0) in calls)
                            ch_hi = max(ch0 + nidx // P
                                        for (_, _, nidx, _, ch0) in calls)
                            ng_all = ch_hi - ch_lo
                            smt = spool.tile([P, ng_all, P], BF, tag="sm",
                                             name=f"sm_{l}_{s}")
                            nc.sync.dma_start(
                                out=smt[:],
                                in_=sm_d[:, ch_lo * P:ch_hi * P])
                            s_tiles = [(ch_lo, ch_hi, smt)]
                            # matmuls per chunk
                            tiles_with_chunks = sorted(
                                {chunk_tile[ci] for ci in range(ch_lo, ch_hi)})
                            agg = {t: ps_agg.tile([P, c.F], FP32, tag="agg",
                                                  name=f"agg{l}_{s}_{t}")
                                   for t in tiles_with_chunks}
                            for b, (g, ch0, nchk) in g_tiles.items():
                                for k in range(nchk):
                                    ci = ch0 + k
                                    t = chunk_tile[ci]
                                    g0s, g1s, sm = next(
                                        (a, b_, smt) for (a, b_, smt) in s_tiles
                                        if a <= ci < b_)
                                    if "nomm" not in ABL:
                                        nc.tensor.matmul(
                                            out=agg[t][:],
                                            lhsT=sm[:, ci - g0s, :],
                                            rhs=g[:, k, :],
                                            start=start_flag[ci],
                                            stop=stop_flag[ci])
                                    elif start_flag[ci]:
                                        nc.tensor.matmul(
                                            out=agg[t][:],
                                            lhsT=sm[:, ci - g0s, :],
                                            rhs=g[:, k, :],
                                            start=True, stop=True)
                            # dense per tile
                            for t in tiles:
                                if t not in agg:
                                    continue
                                x0t2 = tpool.tile([P, c.F], BF, tag="x0r")
                                nc.sync.dma_start(
                                    out=x0t2[:], in_=x0s_hbm[t * P:(t + 1) * P, :])
                                t_sb = tpool.tile([P, c.F], BF, tag="t")
                                nc.vector.tensor_tensor(
                                    out=t_sb[:], in0=agg[t][:], in1=x0t2[:],
                                    op=mybir.AluOpType.add)
                                tT_ps = ps_t.tile([P, c.F], BF, tag="tT")
                                for n in range(c.NETS):
                                    nc.tensor.transpose(
                                        out=tT_ps[:, n * c.H:(n + 1) * c.H],
                                        in_=t_sb[:, n * c.H:(n + 1) * c.H],
                                        identity=idb_sb[:])
                                tT_sb = tpool.tile([P, c.F], BF, tag="tTs")
                                nc.scalar.activation(
                                    out=tT_sb[:], in_=tT_ps[:],
                                    func=mybir.ActivationFunctionType.Copy)
                                u_ps = ps_u.tile([P, c.F], FP32, tag="u")
                                for n in range(c.NETS):
                                    nc.tensor.matmul(
                                        out=u_ps[:, n * c.H:(n + 1) * c.H],
                                        lhsT=tT_sb[:, n * c.H:(n + 1) * c.H],
                                        rhs=cw_sb[:, l * c.F + n * c.H:
                                                  l * c.F + (n + 1) * c.H],
                                        start=True, stop=True)
                                s_sb = tpool.tile([P, c.F], FP32, tag="s")
                                nc.vector.tensor_tensor(
                                    out=s_sb[:], in0=u_ps[:], in1=t_sb[:],
                                    op=mybir.AluOpType.add)
                                h_sb = tpool.tile([P, c.F], BF, tag="h")
                                nc.scalar.activation(
                                    out=h_sb[:], in_=s_sb[:],
                                    func=mybir.ActivationFunctionType.Relu,
                                    scale=float(1.0 - beta))
                                dst_hbm = h4_hbm if l == c.L - 1 else h_shard[pw]
                                nc.sync.dma_start(
                                    out=dst_hbm[t * P:(t + 1) * P, :], in_=h_sb[:])
                        if l < c.L - 1:
                            if "noag" not in ABL:
                                nc.gpsimd.collective_compute(
                                    "AllGather", mybir.AluOpType.bypass,
                                    replica_groups=[list(range(c.n_cores))],
                                    ins=[h_shard[pw][:, :]],
                                    outs=[h_full[l + 1][:, :]])
                            else:
                                nc.sync.dma_start(
                                    out=h_full[l + 1][:c.SHARD, :],
                                    in_=h_shard[pw][:, :])

                # ---------------- Stage C: output layer ----------------
                NO = c.NETS * c.OUT
                with tc.tile_pool(name="sC", bufs=4) as sC, \
                     tc.tile_pool(name="pC", bufs=2, space="PSUM") as pC:
                    for t in real_tiles:
                        h4t = sC.tile([P, c.F], BF, tag="h4")
                        nc.sync.dma_start(out=h4t[:],
                                          in_=h4_hbm[t * P:(t + 1) * P, :])
                        hT_ps = pC.tile([P, c.F], BF, tag="hT")
                        for n in range(c.NETS):
                            nc.tensor.transpose(
                                out=hT_ps[:, n * c.H:(n + 1) * c.H],
                                in_=h4t[:, n * c.H:(n + 1) * c.H],
                                identity=idb_sb[:])
                        hT_sb = sC.tile([P, c.F], BF, tag="hTs")
                        nc.scalar.activation(out=hT_sb[:], in_=hT_ps[:],
                                             func=mybir.ActivationFunctionType.Copy)
                        o_ps = pC.tile([P, NO], FP32, tag="o")
                        for n in range(c.NETS):
                            nc.tensor.matmul(
                                out=o_ps[:, n * c.OUT:(n + 1) * c.OUT],
                                lhsT=hT_sb[:, n * c.H:(n + 1) * c.H],
                                rhs=w1_sb[:, n * c.OUT:(n + 1) * c.OUT],
                                start=True, stop=True)
                        ob = sC.tile([P, NO], FP32, tag="ob")
                        nc.vector.tensor_tensor(out=ob[:], in0=o_ps[:], in1=b1_sb[:],
                                                op=mybir.AluOpType.add)
                        logps = []
                        acc = sC.tile([P, c.OUT], FP32, tag="acc")
                        for n in range(c.NETS):
                            osl = ob[:, n * c.OUT:(n + 1) * c.OUT]
                            nmax = sC.tile([P, 1], FP32, tag=f"nmax{n}")
                            nc.vector.tensor_reduce(
                                out=nmax[:], in_=osl, axis=mybir.AxisListType.X,
                                op=mybir.AluOpType.max, negate=True)
                            e = sC.tile([P, c.OUT], FP32, tag=f"e{n}")
                            nc.scalar.activation(
                                out=e[:], in_=osl,
                                func=mybir.ActivationFunctionType.Exp,
                                bias=nmax[:, :1])
                            ssum = sC.tile([P, 1], FP32, tag=f"ss{n}")
                            nc.vector.tensor_reduce(
                                out=ssum[:], in_=e[:], axis=mybir.AxisListType.X,
                                op=mybir.AluOpType.add)
                            lsum = sC.tile([P, 1], FP32, tag=f"ls{n}")
                            nc.scalar.activation(
                                out=lsum[:], in_=ssum[:],
                                func=mybir.ActivationFunctionType.Ln)
                            lp = sC.tile([P, c.OUT], FP32, tag=f"lp{n}")
                            nc.vector.tensor_scalar(
                                out=lp[:], in0=osl,
                                scalar1=nmax[:, :1], scalar2=lsum[:, :1],
                                op0=mybir.AluOpType.add,
                                op1=mybir.AluOpType.subtract)
                            logps.append(lp)
                        nc.vector.tensor_tensor(out=acc[:], in0=logps[0][:],
                                                in1=logps[1][:],
                                                op=mybir.AluOpType.add)
                        nc.vector.tensor_tensor(out=acc[:], in0=acc[:],
                                                in1=logps[2][:],
                                                op=mybir.AluOpType.add)
                        outt = sC.tile([P, c.OUT], FP32, tag="outt")
                        nc.vector.tensor_scalar(
                            out=outt[:], in0=acc[:], scalar1=1.0 / 3.0,
                            scalar2=None, op0=mybir.AluOpType.mult)
                        nc.sync.dma_start(out=out_d[t * P:(t + 1) * P, :],
                                          in_=outt[:])
    nc.compile()
    return nc


# ----------------------------------------------------------------------------
# Public entry point
# ----------------------------------------------------------------------------
_CACHE = {}


def _get_compiled(reps=1):
    key = ("nc", reps)
    if key not in _CACHE:
        cfg = Cfg(N=100000, IN=128, H=128, OUT=112, L=4, NETS=3,
                  n_cores=8, n_buckets=4, t_sup=4, sg=16, reps=reps)
        _CACHE[key] = (cfg, None)  # placeholder; schedule needed first
    return _CACHE[key]


def kernel(x, edge_index, edge_weight, lin0_w, lin0_b, lin1_w, lin1_b, conv_w):
    """GCN2Conv 3-net ensemble forward on 8 TRN2 NeuronCores.

    Node-sharded: 12544 rows/core (nodes padded to 100352). Edges are
    partitioned by destination; per layer the full [100352, 384] fp16
    activation table (3 nets fused) is AllGathered and source rows are
    fetched with dma_gather. Aggregation runs on the tensor engine via
    on-chip one-hot matrices. Returns [100000, 112] float32 log-probs.
    """
    import numpy as _np
    from concourse.bass_utils import run_bass_kernel_spmd

    cfg = Cfg(N=100000, IN=128, H=128, OUT=112, L=4, NETS=3,
              n_cores=8, n_buckets=4, t_sup=4, sg=16, reps=1)
    in_maps, schedule = host_prep(cfg, x, edge_index, edge_weight,
                                  lin0_w, lin0_b, lin1_w, lin1_b, conv_w)
    skey = (schedule["tot_chunks"], schedule["idx_cols"])
    if _CACHE.get("skey") != skey:
        _CACHE["nc"] = build_nc(cfg, schedule)
        _CACHE["skey"] = skey
    nc = _CACHE["nc"]
    res = run_bass_kernel_spmd(nc, in_maps, core_ids=list(range(cfg.n_cores)))
    out = _np.concatenate([res.results[k]["out"] for k in range(cfg.n_cores)],
                          axis=0)[:cfg.N]
    return _np.ascontiguousarray(out.astype(_np.float32))



# revision 2
# speedup vs baseline: 3.5901x; 3.5901x over previous
"""GCN2 ensemble kernel for Trainium2, 8-way node-sharded.

Design:
  - Nodes padded to N_pad = n_cores * SHARD, SHARD = TPC*128 rows per core.
  - Edges partitioned by dst across cores; within a core grouped by
    (dst tile, src bucket); each group padded to a multiple of 128 "chunk"
    edges (pad idx=0, weight=0 -> no contribution).
  - Chunk schedule (chunks per (tile,bucket)) is the max over cores so the
    single SPMD program fits every core's data.
  - SpMM: dma_gather rows of h (3 nets fused, F=384; fp8e4m3 512B-padded
    rows by default) into G[128 edges, nch, ROW]; S = one-hot(dst)*w
    streamed from DRAM (fp8); PE matmul agg += S.T @ G[:, :, :384].
  - Dense: t = agg + 0.1*x0 (bf16); u = t @ (beta/(1-beta) cw) via PE
    transpose; h' = relu((1-beta)*(t+u)) -> fp8 table row (layers 0-2)
    or bf16 h4 (layer 3).
  - AllGather of the fp8 h shard -> shared h_full per layer.
"""
import dataclasses
import math
import numpy as np
import ml_dtypes

import os as _os
import concourse.bacc as bacc
import concourse.bass as bass
import concourse.mybir as mybir
import concourse.tile as tile

BF16 = np.dtype(np.float16)
FP8 = np.dtype(ml_dtypes.float8_e4m3)
P = 128
ALPHA = 0.1
THETA = 0.5


class Cfg:
    def __init__(self, N=100000, IN=128, H=128, OUT=112, L=4, NETS=3,
                 n_cores=8, n_buckets=4, t_sup=4, sg=16, reps=1,
                 ag_shared=True, fp8=True):
        self.N, self.IN, self.H, self.OUT, self.L, self.NETS = N, IN, H, OUT, L, NETS
        self.n_cores = n_cores
        self.F = NETS * H
        self.fp8 = fp8
        self.ROW = 512 if fp8 else self.F   # table row elements (256B align)
        self.TPC = math.ceil(N / (n_cores * P))       # tiles per core
        self.SHARD = self.TPC * P
        self.N_pad = n_cores * self.SHARD
        self.NB = n_buckets
        assert self.N_pad % n_buckets == 0
        self.BUCKET = self.N_pad // n_buckets
        assert self.BUCKET <= 32768
        self.T_SUP = t_sup
        self.SG = sg                                   # chunks per S-build group
        self.n_sup = math.ceil(self.TPC / t_sup)
        self.reps = reps
        self.ag_shared = ag_shared
        self.betas = [float(np.log(THETA / (l + 1) + 1.0)) for l in range(L)]


def host_prep(cfg, x, edge_index, edge_weight,
              lin0_w, lin0_b, lin1_w, lin1_b, conv_w):
    """Build per-core input maps + the static chunk schedule."""
    c = cfg
    SDT = FP8 if c.fp8 else BF16
    src = np.asarray(edge_index[0], dtype=np.int64)
    dst = np.asarray(edge_index[1], dtype=np.int64)
    w = np.asarray(edge_weight, dtype=np.float32)

    core = dst // c.SHARD
    tloc = (dst % c.SHARD) // P
    dloc = (dst % P).astype(np.int32)
    buck = src // c.BUCKET
    sloc = (src % c.BUCKET).astype(np.int32)

    # counts per (core, tile, bucket)
    cnt = np.zeros((c.n_cores, c.TPC, c.NB), np.int64)
    np.add.at(cnt, (core, tloc, buck), 1)
    nch = np.ceil(cnt / P).astype(np.int64).max(axis=0)  # [TPC, NB]
    # real tiles with zero chunks still need one dummy chunk so PSUM is written
    for t in range(c.TPC):
        if nch[t].sum() == 0:
            if any(k * c.SHARD + t * P < c.N for k in range(c.n_cores)):
                nch[t][0] = 1

    # schedule: for sup, for b, for t in sup: nch[t][b] chunks
    sup_tiles = [list(range(s * c.T_SUP, min((s + 1) * c.T_SUP, c.TPC)))
                 for s in range(c.n_sup)]
    chunk_tile = []        # global chunk idx -> tile (core-local)
    call_list = []         # (sup, b, nidx, idx_col_off, chunk0)
    idx_col_off = 0
    for s, tiles in enumerate(sup_tiles):
        for b in range(c.NB):
            ncall = int(sum(nch[t][b] for t in tiles))
            if ncall == 0:
                continue
            call_list.append((s, b, ncall * P, idx_col_off, len(chunk_tile)))
            idx_col_off += ncall * P // 16
            for t in tiles:
                chunk_tile.extend([t] * int(nch[t][b]))
    tot_chunks = len(chunk_tile)
    idx_cols = idx_col_off

    # start/stop flags per chunk (per tile, over the whole schedule)
    first_chunk = {}
    last_chunk = {}
    for ci, t in enumerate(chunk_tile):
        first_chunk.setdefault(t, ci)
        last_chunk[t] = ci
    start_flag = [first_chunk[t] == ci for ci, t in enumerate(chunk_tile)]
    stop_flag = [last_chunk[t] == ci for ci, t in enumerate(chunk_tile)]

    group_chunk0 = {}
    ci = 0
    for s, tiles in enumerate(sup_tiles):
        for b in range(c.NB):
            if sum(nch[t][b] for t in tiles) == 0:
                continue
            for t in tiles:
                if nch[t][b] > 0:
                    group_chunk0[(t, b)] = ci
                    ci += int(nch[t][b])
    assert ci == tot_chunks

    # per-core arrays
    order = np.lexsort((sloc, buck, tloc, core))  # sort edges
    src_s, w_s = sloc[order], w[order]
    core_s, tloc_s, buck_s, dloc_s = core[order], tloc[order], buck[order], dloc[order]

    schedule = dict(call_list=call_list, chunk_tile=chunk_tile,
                    start_flag=start_flag, stop_flag=stop_flag,
                    tot_chunks=tot_chunks, idx_cols=idx_cols,
                    sup_tiles=sup_tiles, nch=nch)

    in_maps = []
    # weights (shared across cores)
    w0cat = np.ascontiguousarray(
        np.concatenate([lin0_w[n] for n in range(c.NETS)], axis=1)).astype(BF16)
    b0rep = np.tile(np.concatenate([lin0_b[n] for n in range(c.NETS)])[None, :],
                    (P, 1)).astype(np.float32)
    w1cat = np.ascontiguousarray(
        np.concatenate([lin1_w[n] for n in range(c.NETS)], axis=1)).astype(BF16)
    b1rep = np.tile(np.concatenate([lin1_b[n] for n in range(c.NETS)])[None, :],
                    (P, 1)).astype(np.float32)
    cwcat = np.zeros((c.L, c.H, c.F), np.float32)
    for l in range(c.L):
        beta = c.betas[l]
        for n in range(c.NETS):
            cwcat[l][:, n * c.H:(n + 1) * c.H] = conv_w[n, l] * (beta / (1.0 - beta))
    cwcat = cwcat.astype(BF16)
    ident_b = np.eye(P, dtype=np.float32).astype(BF16)
    ident_f = np.eye(P, dtype=np.float32)

    for k in range(c.n_cores):
        msk = core_s == k
        idx_flat = np.zeros(tot_chunks * P, np.int32)
        dst_flat = np.zeros(tot_chunks * P, np.int32)
        wv_flat = np.zeros(tot_chunks * P, np.float32)
        tl, bk = tloc_s[msk], buck_s[msk]
        gkey = tl * c.NB + bk
        gkeys, counts = np.unique(gkey, return_counts=True)
        pos_in_group = np.arange(len(gkey)) - np.repeat(
            np.concatenate([[0], np.cumsum(counts)[:-1]]), counts)
        group_base = np.array([group_chunk0[(gk // c.NB, gk % c.NB)] * P
                               for gk in gkeys])
        base_per_edge = np.repeat(group_base, counts)
        pos = base_per_edge + pos_in_group
        idx_flat[pos] = src_s[msk]
        dst_flat[pos] = dloc_s[msk]
        wv_flat[pos] = w_s[msk] * (1.0 - ALPHA)

        # wrap idx per call into [128, idx_cols] int16
        idx_all = np.zeros((P, idx_cols), np.int16)
        for (s, b, nidx, coloff, ch0) in call_list:
            a = idx_flat[ch0 * P: ch0 * P + nidx]
            wrapped = a.reshape(nidx // 16, 16).T.astype(np.int16)
            idx_all[:, coloff:coloff + nidx // 16] = np.tile(wrapped, (8, 1))
        # precomputed one-hot*weight S: sm[p, ci*128 + d] = w_edge(ci*128+p)
        allpos = np.arange(tot_chunks * P)
        sm_all = np.zeros((P, tot_chunks * P), SDT)
        sm_all[allpos % P, (allpos // P) * P + dst_flat] = wv_flat.astype(SDT)

        xs = np.zeros((c.SHARD, c.IN), np.float32)
        lo, hi = k * c.SHARD, min((k + 1) * c.SHARD, c.N)
        if hi > lo:
            xs[:hi - lo] = np.asarray(x[lo:hi], np.float32)

        in_maps.append({
            "xs": xs, "idx_all": idx_all, "sm_all": sm_all,
            "w0cat": w0cat, "b0rep": b0rep, "w1cat": w1cat, "b1rep": b1rep,
            "cwcat": cwcat, "ident_b": ident_b, "ident_f": ident_f,
        })
    return in_maps, schedule


def build_nc(cfg, schedule):
    c = cfg
    ABL = set(_os.environ.get("GCN_ABLATE", "").split(","))
    single_packet = _os.environ.get("GCN_SP", "0") == "1"
    nch = schedule["nch"]
    call_list = schedule["call_list"]
    chunk_tile = schedule["chunk_tile"]
    start_flag = schedule["start_flag"]
    stop_flag = schedule["stop_flag"]
    sup_tiles = schedule["sup_tiles"]
    idx_cols = schedule["idx_cols"]
    tot_chunks = schedule["tot_chunks"]
    FP32 = mybir.dt.float32
    BF = mybir.dt.float16
    F8 = mybir.dt.float8e4
    HDT = F8 if c.fp8 else BF    # h-table / gather / S dtype
    ROW = c.ROW

    nc = bacc.Bacc("TRN2", target_bir_lowering=False, debug=False,
                   num_devices=c.n_cores, num_swdge_queues=4)
    # I/O
    xs = nc.dram_tensor("xs", [c.SHARD, c.IN], FP32, kind="ExternalInput").ap()
    idx_all_d = nc.dram_tensor("idx_all", [P, idx_cols], mybir.dt.int16,
                               kind="ExternalInput").ap()
    sm_d = nc.dram_tensor("sm_all", [P, tot_chunks * P], HDT,
                          kind="ExternalInput").ap()
    w0cat_d = nc.dram_tensor("w0cat", [c.IN, c.F], BF, kind="ExternalInput").ap()
    b0rep_d = nc.dram_tensor("b0rep", [P, c.F], FP32, kind="ExternalInput").ap()
    w1cat_d = nc.dram_tensor("w1cat", [c.H, c.NETS * c.OUT], BF,
                             kind="ExternalInput").ap()
    b1rep_d = nc.dram_tensor("b1rep", [P, c.NETS * c.OUT], FP32,
                             kind="ExternalInput").ap()
    cwcat_d = nc.dram_tensor("cwcat", [c.L, c.H, c.F], BF, kind="ExternalInput").ap()
    identb_d = nc.dram_tensor("ident_b", [P, P], BF, kind="ExternalInput").ap()
    identf_d = nc.dram_tensor("ident_f", [P, P], FP32, kind="ExternalInput").ap()
    out_d = nc.dram_tensor("out", [c.SHARD, c.OUT], FP32, kind="ExternalOutput").ap()

    with tile.TileContext(nc) as tc:
        with tc.tile_pool(name="dram", bufs=1, space="DRAM") as dram, \
             tc.tile_pool(name="const", bufs=1) as cst:
            h_shard = [dram.tile([c.SHARD, ROW], HDT, name=f"h_shard{p}")
                       for p in range(2)]
            h_full_space = "Shared" if (c.n_cores > 4 and c.ag_shared) else "Local"
            h_full_all = [[dram.tile([c.N_pad, ROW], HDT,
                                     addr_space=h_full_space,
                                     name=f"h_full_r{r}_l{l}")
                           for l in range(c.L)] for r in range(c.reps)]
            x0s_hbm = dram.tile([c.SHARD, c.F], BF, name="x0s_hbm")
            h4_hbm = dram.tile([c.SHARD, c.F], BF, name="h4_hbm")

            # resident constants
            w0_sb = cst.tile([P, c.F], BF)
            b0_sb = cst.tile([P, c.F], FP32)
            w1_sb = cst.tile([P, c.NETS * c.OUT], BF)
            b1_sb = cst.tile([P, c.NETS * c.OUT], FP32)
            cw_sb = cst.tile([P, c.L * c.F], BF)
            idb_sb = cst.tile([P, P], BF)
            idf_sb = cst.tile([P, P], FP32)
            nc.sync.dma_start(out=w0_sb[:], in_=w0cat_d)
            nc.sync.dma_start(out=b0_sb[:], in_=b0rep_d)
            nc.sync.dma_start(out=w1_sb[:], in_=w1cat_d)
            nc.sync.dma_start(out=b1_sb[:], in_=b1rep_d)
            for l in range(c.L):
                nc.sync.dma_start(out=cw_sb[:, l * c.F:(l + 1) * c.F], in_=cwcat_d[l])
            nc.sync.dma_start(out=idb_sb[:], in_=identb_d)
            nc.sync.dma_start(out=idf_sb[:], in_=identf_d)

            def tile_is_real(t):
                return any(k * c.SHARD + t * P < c.N for k in range(c.n_cores))

            real_tiles = [t for t in range(c.TPC) if tile_is_real(t)]

            serial_reps = "serial" in ABL
            for rep in range(c.reps):
                # ---------------- Stage A: input layer ----------------
                with tc.tile_pool(name="sA", bufs=4) as sA, \
                     tc.tile_pool(name="pA", bufs=2, space="PSUM") as pA:
                    gate = None
                    if serial_reps and rep > 0:
                        od = sA.tile([P, c.OUT], FP32, tag="od", name=f"od{rep}")
                        nc.sync.dma_start(out=od[:], in_=out_d[0:P, :])
                        g0 = sA.tile([P, 1], FP32, tag="g0", name=f"g0{rep}")
                        nc.vector.tensor_reduce(
                            out=g0[:], in_=od[:], axis=mybir.AxisListType.X,
                            op=mybir.AluOpType.max)
                        gate = sA.tile([P, 1], FP32, tag="g1", name=f"g1{rep}")
                        nc.vector.tensor_scalar(
                            out=gate[:], in0=g0[:], scalar1=0.0, scalar2=1.0,
                            op0=mybir.AluOpType.mult, op1=mybir.AluOpType.add)
                    for t in real_tiles:
                        xt = sA.tile([P, c.IN], FP32, tag="xt")
                        nc.sync.dma_start(out=xt[:], in_=xs[t * P:(t + 1) * P, :])
                        if gate is not None:
                            xg = sA.tile([P, c.IN], FP32, tag="xg")
                            nc.vector.tensor_tensor(
                                out=xg[:], in0=xt[:],
                                in1=gate[:, 0:1].to_broadcast([P, c.IN]),
                                op=mybir.AluOpType.mult)
                            xt = xg
                        xT_ps = pA.tile([P, P], FP32, tag="xT")
                        nc.tensor.transpose(out=xT_ps[:], in_=xt[:], identity=idf_sb[:])
                        xT_sb = sA.tile([P, P], BF, tag="xTs")
                        nc.scalar.activation(out=xT_sb[:], in_=xT_ps[:],
                                             func=mybir.ActivationFunctionType.Copy)
                        h0_ps = pA.tile([P, c.F], FP32, tag="h0")
                        nc.tensor.matmul(out=h0_ps[:], lhsT=xT_sb[:], rhs=w0_sb[:],
                                         start=True, stop=True)
                        hb = sA.tile([P, c.F], FP32, tag="hb")
                        nc.vector.tensor_tensor(out=hb[:], in0=h0_ps[:], in1=b0_sb[:],
                                                op=mybir.AluOpType.add)
                        h0t = sA.tile([P, c.F], BF, tag="h0t")
                        nc.scalar.activation(out=h0t[:], in_=hb[:],
                                             func=mybir.ActivationFunctionType.Relu)
                        x0t = sA.tile([P, c.F], BF, tag="x0t")
                        nc.vector.tensor_scalar(out=x0t[:], in0=h0t[:],
                                                scalar1=ALPHA, scalar2=None,
                                                op0=mybir.AluOpType.mult)
                        if c.fp8:
                            hq = sA.tile([P, c.F], F8, tag="hq")
                            nc.scalar.activation(
                                out=hq[:], in_=hb[:],
                                func=mybir.ActivationFunctionType.Relu)
                        else:
                            hq = h0t
                        nc.sync.dma_start(
                            out=h_shard[0][t * P:(t + 1) * P, 0:c.F], in_=hq[:])
                        nc.sync.dma_start(
                            out=x0s_hbm[t * P:(t + 1) * P, :], in_=x0t[:])

                # AG0
                h_full = h_full_all[rep]
                if "noag" not in ABL:
                    nc.gpsimd.collective_compute(
                        "AllGather", mybir.AluOpType.bypass,
                        replica_groups=[list(range(c.n_cores))],
                        ins=[h_shard[0][:, :]], outs=[h_full[0][:, :]])
                else:
                    nc.sync.dma_start(
                        out=h_full[0][:c.SHARD, :], in_=h_shard[0][:, :])

                # ---------------- Stage B: GCN2 layers ----------------
                with tc.tile_pool(name="gpool", bufs=2) as gpool, \
                     tc.tile_pool(name="spool", bufs=2) as spool, \
                     tc.tile_pool(name="tpool", bufs=6) as tpool, \
                     tc.tile_pool(name="psA", bufs=4, space="PSUM") as ps_agg, \
                     tc.tile_pool(name="psT", bufs=2, space="PSUM") as ps_t, \
                     tc.tile_pool(name="psU", bufs=2, space="PSUM") as ps_u:
                    for l in range(c.L):
                        pr, pw = l % 2, (l + 1) % 2
                        beta = c.betas[l]
                        gather_cnt = 0
                        for s, tiles in enumerate(sup_tiles):
                            calls = [cl for cl in call_list if cl[0] == s]
                            if not calls:
                                continue
                            g_tiles = {}
                            col_lo = min(cl[3] for cl in calls)
                            col_hi = max(cl[3] + cl[2] // 16 for cl in calls)
                            ix = spool.tile([P, col_hi - col_lo],
                                            mybir.dt.int16, tag="ix",
                                            name=f"ix_{l}_{s}")
                            nc.sync.dma_start(
                                out=ix[:], in_=idx_all_d[:, col_lo:col_hi])
                            for (s_, b, nidx, coloff, ch0) in calls:
                                nchk = nidx // P
                                g = gpool.tile([P, nchk, ROW], HDT, tag=f"g{b}")
                                if "nogather" in ABL:
                                    nc.vector.memset(g[:], 0)
                                if "nogather" not in ABL:
                                    nc.gpsimd.dma_gather(
                                        g[:],
                                        h_full[l][b * c.BUCKET:(b + 1) * c.BUCKET, :],
                                        ix[:, coloff - col_lo:
                                           coloff - col_lo + nidx // 16],
                                        nidx, nidx, ROW,
                                        single_packet=single_packet,
                                        queue_num=gather_cnt % 4)
                                gather_cnt += 1
                                g_tiles[b] = (g, ch0, nchk)
                            ch_lo = min(ch0 for (_, _, _, _, ch0) in calls)
                            ch_hi = max(ch0 + nidx // P
                                        for (_, _, nidx, _, ch0) in calls)
                            ng_all = ch_hi - ch_lo
                            smt = spool.tile([P, ng_all, P], HDT, tag="sm",
                                             name=f"sm_{l}_{s}")
                            nc.sync.dma_start(
                                out=smt[:],
                                in_=sm_d[:, ch_lo * P:ch_hi * P])
                            s_tiles = [(ch_lo, ch_hi, smt)]
                            tiles_with_chunks = sorted(
                                {chunk_tile[ci] for ci in range(ch_lo, ch_hi)})
                            agg = {t: ps_agg.tile([P, c.F], FP32, tag="agg",
                                                  name=f"agg{l}_{s}_{t}")
                                   for t in tiles_with_chunks}
                            for b, (g, ch0, nchk) in g_tiles.items():
                                for k in range(nchk):
                                    ci = ch0 + k
                                    t = chunk_tile[ci]
                                    g0s, g1s, sm = next(
                                        (a, b_, smt) for (a, b_, smt) in s_tiles
                                        if a <= ci < b_)
                                    if "nomm" not in ABL:
                                        nc.tensor.matmul(
                                            out=agg[t][:],
                                            lhsT=sm[:, ci - g0s, :],
                                            rhs=g[:, k, 0:c.F],
                                            start=start_flag[ci],
                                            stop=stop_flag[ci])
                                    elif start_flag[ci]:
                                        nc.tensor.matmul(
                                            out=agg[t][:],
                                            lhsT=sm[:, ci - g0s, :],
                                            rhs=g[:, k, 0:c.F],
                                            start=True, stop=True)
                            # dense per tile
                            for t in tiles:
                                if t not in agg:
                                    continue
                                x0t2 = tpool.tile([P, c.F], BF, tag="x0r")
                                nc.sync.dma_start(
                                    out=x0t2[:], in_=x0s_hbm[t * P:(t + 1) * P, :])
                                t_sb = tpool.tile([P, c.F], BF, tag="t")
                                nc.vector.tensor_tensor(
                                    out=t_sb[:], in0=agg[t][:], in1=x0t2[:],
                                    op=mybir.AluOpType.add)
                                tT_ps = ps_t.tile([P, c.F], BF, tag="tT")
                                for n in range(c.NETS):
                                    nc.tensor.transpose(
                                        out=tT_ps[:, n * c.H:(n + 1) * c.H],
                                        in_=t_sb[:, n * c.H:(n + 1) * c.H],
                                        identity=idb_sb[:])
                                tT_sb = tpool.tile([P, c.F], BF, tag="tTs")
                                nc.scalar.activation(
                                    out=tT_sb[:], in_=tT_ps[:],
                                    func=mybir.ActivationFunctionType.Copy)
                                u_ps = ps_u.tile([P, c.F], FP32, tag="u")
                                for n in range(c.NETS):
                                    nc.tensor.matmul(
                                        out=u_ps[:, n * c.H:(n + 1) * c.H],
                                        lhsT=tT_sb[:, n * c.H:(n + 1) * c.H],
                                        rhs=cw_sb[:, l * c.F + n * c.H:
                                                  l * c.F + (n + 1) * c.H],
                                        start=True, stop=True)
                                s_sb = tpool.tile([P, c.F], FP32, tag="s")
                                nc.vector.tensor_tensor(
                                    out=s_sb[:], in0=u_ps[:], in1=t_sb[:],
                                    op=mybir.AluOpType.add)
                                if l == c.L - 1:
                                    h_sb = tpool.tile([P, c.F], BF, tag="h")
                                    nc.scalar.activation(
                                        out=h_sb[:], in_=s_sb[:],
                                        func=mybir.ActivationFunctionType.Relu,
                                        scale=float(1.0 - beta))
                                    nc.sync.dma_start(
                                        out=h4_hbm[t * P:(t + 1) * P, :],
                                        in_=h_sb[:])
                                else:
                                    h_sb = tpool.tile([P, c.F], HDT, tag="h8")
                                    nc.scalar.activation(
                                        out=h_sb[:], in_=s_sb[:],
                                        func=mybir.ActivationFunctionType.Relu,
                                        scale=float(1.0 - beta))
                                    nc.sync.dma_start(
                                        out=h_shard[pw][t * P:(t + 1) * P, 0:c.F],
                                        in_=h_sb[:])
                        if l < c.L - 1:
                            if "noag" not in ABL:
                                nc.gpsimd.collective_compute(
                                    "AllGather", mybir.AluOpType.bypass,
                                    replica_groups=[list(range(c.n_cores))],
                                    ins=[h_shard[pw][:, :]],
                                    outs=[h_full[l + 1][:, :]])
                            else:
                                nc.sync.dma_start(
                                    out=h_full[l + 1][:c.SHARD, :],
                                    in_=h_shard[pw][:, :])

                # ---------------- Stage C: output layer ----------------
                NO = c.NETS * c.OUT
                with tc.tile_pool(name="sC", bufs=4) as sC, \
                     tc.tile_pool(name="pC", bufs=2, space="PSUM") as pC:
                    for t in real_tiles:
                        h4t = sC.tile([P, c.F], BF, tag="h4")
                        nc.sync.dma_start(out=h4t[:],
                                          in_=h4_hbm[t * P:(t + 1) * P, :])
                        hT_ps = pC.tile([P, c.F], BF, tag="hT")
                        for n in range(c.NETS):
                            nc.tensor.transpose(
                                out=hT_ps[:, n * c.H:(n + 1) * c.H],
                                in_=h4t[:, n * c.H:(n + 1) * c.H],
                                identity=idb_sb[:])
                        hT_sb = sC.tile([P, c.F], BF, tag="hTs")
                        nc.scalar.activation(out=hT_sb[:], in_=hT_ps[:],
                                             func=mybir.ActivationFunctionType.Copy)
                        o_ps = pC.tile([P, NO], FP32, tag="o")
                        for n in range(c.NETS):
                            nc.tensor.matmul(
                                out=o_ps[:, n * c.OUT:(n + 1) * c.OUT],
                                lhsT=hT_sb[:, n * c.H:(n + 1) * c.H],
                                rhs=w1_sb[:, n * c.OUT:(n + 1) * c.OUT],
                                start=True, stop=True)
                        ob = sC.tile([P, NO], FP32, tag="ob")
                        nc.vector.tensor_tensor(out=ob[:], in0=o_ps[:], in1=b1_sb[:],
                                                op=mybir.AluOpType.add)
                        logps = []
                        acc = sC.tile([P, c.OUT], FP32, tag="acc")
                        for n in range(c.NETS):
                            osl = ob[:, n * c.OUT:(n + 1) * c.OUT]
                            nmax = sC.tile([P, 1], FP32, tag=f"nmax{n}")
                            nc.vector.tensor_reduce(
                                out=nmax[:], in_=osl, axis=mybir.AxisListType.X,
                                op=mybir.AluOpType.max, negate=True)
                            e = sC.tile([P, c.OUT], FP32, tag=f"e{n}")
                            nc.scalar.activation(
                                out=e[:], in_=osl,
                                func=mybir.ActivationFunctionType.Exp,
                                bias=nmax[:, :1])
                            ssum = sC.tile([P, 1], FP32, tag=f"ss{n}")
                            nc.vector.tensor_reduce(
                                out=ssum[:], in_=e[:], axis=mybir.AxisListType.X,
                                op=mybir.AluOpType.add)
                            lsum = sC.tile([P, 1], FP32, tag=f"ls{n}")
                            nc.scalar.activation(
                                out=lsum[:], in_=ssum[:],
                                func=mybir.ActivationFunctionType.Ln)
                            lp = sC.tile([P, c.OUT], FP32, tag=f"lp{n}")
                            nc.vector.tensor_scalar(
                                out=lp[:], in0=osl,
                                scalar1=nmax[:, :1], scalar2=lsum[:, :1],
                                op0=mybir.AluOpType.add,
                                op1=mybir.AluOpType.subtract)
                            logps.append(lp)
                        nc.vector.tensor_tensor(out=acc[:], in0=logps[0][:],
                                                in1=logps[1][:],
                                                op=mybir.AluOpType.add)
                        nc.vector.tensor_tensor(out=acc[:], in0=acc[:],
                                                in1=logps[2][:],
                                                op=mybir.AluOpType.add)
                        outt = sC.tile([P, c.OUT], FP32, tag="outt")
                        nc.vector.tensor_scalar(
                            out=outt[:], in0=acc[:], scalar1=1.0 / 3.0,
                            scalar2=None, op0=mybir.AluOpType.mult)
                        nc.sync.dma_start(out=out_d[t * P:(t + 1) * P, :],
                                          in_=outt[:])
    nc.compile()
    return nc


# ----------------------------------------------------------------------------
# Public entry point
# ----------------------------------------------------------------------------
_CACHE = {}


def kernel(x, edge_index, edge_weight, lin0_w, lin0_b, lin1_w, lin1_b, conv_w):
    """GCN2Conv 3-net ensemble forward on 8 TRN2 NeuronCores.

    Node-sharded: 12544 rows/core (nodes padded to 100352). Edges are
    partitioned by destination; per layer the full [100352, 512] fp8
    activation table (3 nets fused, 512B-aligned rows) is AllGathered and
    source rows are fetched with dma_gather. Aggregation runs on the tensor
    engine via streamed one-hot matrices. Returns [100000, 112] float32
    log-probs.
    """
    import numpy as _np
    from concourse.bass_utils import run_bass_kernel_spmd

    cfg = Cfg(N=100000, IN=128, H=128, OUT=112, L=4, NETS=3,
              n_cores=8, n_buckets=4, t_sup=4, sg=16, reps=1)
    in_maps, schedule = host_prep(cfg, x, edge_index, edge_weight,
                                  lin0_w, lin0_b, lin1_w, lin1_b, conv_w)
    skey = (schedule["tot_chunks"], schedule["idx_cols"], cfg.fp8)
    if _CACHE.get("skey") != skey:
        _CACHE["nc"] = build_nc(cfg, schedule)
        _CACHE["skey"] = skey
    nc = _CACHE["nc"]
    res = run_bass_kernel_spmd(nc, in_maps, core_ids=list(range(cfg.n_cores)))
    out = _np.concatenate([res.results[k]["out"] for k in range(cfg.n_cores)],
                          axis=0)[:cfg.N]
    return _np.ascontiguousarray(out.astype(_np.float32))


# revision 5
# speedup vs baseline: 11.8369x; 3.2971x over previous
"""GCN2 ensemble kernel for Trainium2, 8-way node-sharded.

Design:
  - Nodes padded to N_pad = n_cores * SHARD, SHARD = TPC*128 rows per core.
  - Edges partitioned by dst across cores; within a core grouped by
    (dst tile, src bucket); each group padded to a multiple of 128 "chunk"
    edges (pad idx=0, weight=0 -> no contribution).
  - Chunk schedule (chunks per (tile,bucket)) is the max over cores so the
    single SPMD program fits every core's data.
  - SpMM: dma_gather rows of h (3 nets fused, F=384; fp8e4m3 512B-padded
    rows by default) into G[128 edges, nch, ROW]; S = one-hot(dst)*w
    streamed from DRAM (fp8); PE matmul agg += S.T @ G[:, :, :384].
  - Dense: t = agg + 0.1*x0 (bf16); u = t @ (beta/(1-beta) cw) via PE
    transpose; h' = relu((1-beta)*(t+u)) -> fp8 table row (layers 0-2)
    or bf16 h4 (layer 3).
  - AllGather of the fp8 h shard -> shared h_full per layer.
"""
import dataclasses
import math
import numpy as np
import ml_dtypes

import os as _os
import concourse.bacc as bacc
import concourse.bass as bass
import concourse.mybir as mybir
import concourse.tile as tile

BF16 = np.dtype(np.float16)
FP8 = np.dtype(ml_dtypes.float8_e4m3)
P = 128
ALPHA = 0.1
THETA = 0.5


class Cfg:
    def __init__(self, N=100000, IN=128, H=128, OUT=112, L=4, NETS=3,
                 n_cores=8, n_buckets=4, t_sup=6, sg=16, reps=1,
                 ag_shared=True, fp8=True):
        self.N, self.IN, self.H, self.OUT, self.L, self.NETS = N, IN, H, OUT, L, NETS
        self.n_cores = n_cores
        self.F = NETS * H
        self.fp8 = fp8
        self.ROW = 512 if fp8 else self.F   # table row elements (256B align)
        self.TPC = math.ceil(N / (n_cores * P))       # tiles per core
        self.SHARD = self.TPC * P
        self.N_pad = n_cores * self.SHARD
        self.NB = n_buckets
        assert self.N_pad % n_buckets == 0
        self.BUCKET = self.N_pad // n_buckets
        assert self.BUCKET <= 32768
        self.T_SUP = t_sup
        self.SG = sg                                   # chunks per S-build group
        self.n_sup = math.ceil(self.TPC / t_sup)
        self.reps = reps
        self.ag_shared = ag_shared
        self.betas = [float(np.log(THETA / (l + 1) + 1.0)) for l in range(L)]


def host_prep(cfg, x, edge_index, edge_weight,
              lin0_w, lin0_b, lin1_w, lin1_b, conv_w):
    """Build per-core input maps + the static chunk schedule."""
    c = cfg
    SDT = FP8 if c.fp8 else BF16
    src = np.asarray(edge_index[0], dtype=np.int64)
    dst = np.asarray(edge_index[1], dtype=np.int64)
    w = np.asarray(edge_weight, dtype=np.float32)

    core = dst // c.SHARD
    tloc = (dst % c.SHARD) // P
    dloc = (dst % P).astype(np.int32)
    buck = src // c.BUCKET
    sloc = (src % c.BUCKET).astype(np.int32)

    # counts per (core, tile, bucket)
    cnt = np.zeros((c.n_cores, c.TPC, c.NB), np.int64)
    np.add.at(cnt, (core, tloc, buck), 1)
    nch = np.ceil(cnt / P).astype(np.int64).max(axis=0)  # [TPC, NB]
    # real tiles with zero chunks still need one dummy chunk so PSUM is written
    for t in range(c.TPC):
        if nch[t].sum() == 0:
            if any(k * c.SHARD + t * P < c.N for k in range(c.n_cores)):
                nch[t][0] = 1

    # schedule: for sup, for b, for t in sup: nch[t][b] chunks
    sup_tiles = [list(range(s * c.T_SUP, min((s + 1) * c.T_SUP, c.TPC)))
                 for s in range(c.n_sup)]
    chunk_tile = []        # global chunk idx -> tile (core-local)
    call_list = []         # (sup, b, nidx, idx_col_off, chunk0)
    idx_col_off = 0
    for s, tiles in enumerate(sup_tiles):
        for b in range(c.NB):
            ncall = int(sum(nch[t][b] for t in tiles))
            if ncall == 0:
                continue
            call_list.append((s, b, ncall * P, idx_col_off, len(chunk_tile)))
            idx_col_off += ncall * P // 16
            for t in tiles:
                chunk_tile.extend([t] * int(nch[t][b]))
    tot_chunks = len(chunk_tile)
    idx_cols = idx_col_off

    # start/stop flags per chunk (per tile, over the whole schedule)
    first_chunk = {}
    last_chunk = {}
    for ci, t in enumerate(chunk_tile):
        first_chunk.setdefault(t, ci)
        last_chunk[t] = ci
    start_flag = [first_chunk[t] == ci for ci, t in enumerate(chunk_tile)]
    stop_flag = [last_chunk[t] == ci for ci, t in enumerate(chunk_tile)]

    group_chunk0 = {}
    ci = 0
    for s, tiles in enumerate(sup_tiles):
        for b in range(c.NB):
            if sum(nch[t][b] for t in tiles) == 0:
                continue
            for t in tiles:
                if nch[t][b] > 0:
                    group_chunk0[(t, b)] = ci
                    ci += int(nch[t][b])
    assert ci == tot_chunks

    # per-core arrays
    order = np.lexsort((sloc, buck, tloc, core))  # sort edges
    src_s, w_s = sloc[order], w[order]
    core_s, tloc_s, buck_s, dloc_s = core[order], tloc[order], buck[order], dloc[order]

    schedule = dict(call_list=call_list, chunk_tile=chunk_tile,
                    start_flag=start_flag, stop_flag=stop_flag,
                    tot_chunks=tot_chunks, idx_cols=idx_cols,
                    sup_tiles=sup_tiles, nch=nch)

    in_maps = []
    # weights (shared across cores)
    w0cat = np.ascontiguousarray(
        np.concatenate([lin0_w[n] for n in range(c.NETS)], axis=1)).astype(BF16)
    b0rep = np.tile(np.concatenate([lin0_b[n] for n in range(c.NETS)])[None, :],
                    (P, 1)).astype(np.float32)
    w1cat = np.ascontiguousarray(
        np.concatenate([lin1_w[n] for n in range(c.NETS)], axis=1)).astype(BF16)
    b1rep = np.tile(np.concatenate([lin1_b[n] for n in range(c.NETS)])[None, :],
                    (P, 1)).astype(np.float32)
    cwcat = np.zeros((c.L, c.H, c.F), np.float32)
    for l in range(c.L):
        beta = c.betas[l]
        for n in range(c.NETS):
            cwcat[l][:, n * c.H:(n + 1) * c.H] = conv_w[n, l] * (beta / (1.0 - beta))
    cwcat = cwcat.astype(BF16)
    ident_b = np.eye(P, dtype=np.float32).astype(BF16)
    ident_f = np.eye(P, dtype=np.float32)

    for k in range(c.n_cores):
        msk = core_s == k
        idx_flat = np.zeros(tot_chunks * P, np.int32)
        dst_flat = np.zeros(tot_chunks * P, np.int32)
        wv_flat = np.zeros(tot_chunks * P, np.float32)
        tl, bk = tloc_s[msk], buck_s[msk]
        gkey = tl * c.NB + bk
        gkeys, counts = np.unique(gkey, return_counts=True)
        pos_in_group = np.arange(len(gkey)) - np.repeat(
            np.concatenate([[0], np.cumsum(counts)[:-1]]), counts)
        group_base = np.array([group_chunk0[(gk // c.NB, gk % c.NB)] * P
                               for gk in gkeys])
        base_per_edge = np.repeat(group_base, counts)
        pos = base_per_edge + pos_in_group
        idx_flat[pos] = src_s[msk]
        dst_flat[pos] = dloc_s[msk]
        wv_flat[pos] = w_s[msk] * (1.0 - ALPHA)

        # wrap idx per call into [128, idx_cols] int16
        idx_all = np.zeros((P, idx_cols), np.int16)
        for (s, b, nidx, coloff, ch0) in call_list:
            a = idx_flat[ch0 * P: ch0 * P + nidx]
            wrapped = a.reshape(nidx // 16, 16).T.astype(np.int16)
            idx_all[:, coloff:coloff + nidx // 16] = np.tile(wrapped, (8, 1))
        # precomputed one-hot*weight S: sm[p, ci*128 + d] = w_edge(ci*128+p)
        allpos = np.arange(tot_chunks * P)
        sm_all = np.zeros((P, tot_chunks * P), SDT)
        sm_all[allpos % P, (allpos // P) * P + dst_flat] = wv_flat.astype(SDT)

        xs = np.zeros((c.SHARD, c.IN), np.float32)
        lo, hi = k * c.SHARD, min((k + 1) * c.SHARD, c.N)
        if hi > lo:
            xs[:hi - lo] = np.asarray(x[lo:hi], np.float32)

        in_maps.append({
            "xs": xs, "idx_all": idx_all, "sm_all": sm_all,
            "w0cat": w0cat, "b0rep": b0rep, "w1cat": w1cat, "b1rep": b1rep,
            "cwcat": cwcat, "ident_b": ident_b, "ident_f": ident_f,
        })
    return in_maps, schedule


def build_nc(cfg, schedule):
    c = cfg
    ABL = set(_os.environ.get("GCN_ABLATE", "").split(","))
    single_packet = _os.environ.get("GCN_SP", "0") == "1"
    nch = schedule["nch"]
    call_list = schedule["call_list"]
    chunk_tile = schedule["chunk_tile"]
    start_flag = schedule["start_flag"]
    stop_flag = schedule["stop_flag"]
    sup_tiles = schedule["sup_tiles"]
    idx_cols = schedule["idx_cols"]
    tot_chunks = schedule["tot_chunks"]
    FP32 = mybir.dt.float32
    BF = mybir.dt.float16
    F8 = mybir.dt.float8e4
    HDT = F8 if c.fp8 else BF    # h-table / gather / S dtype
    ROW = c.ROW

    nc = bacc.Bacc("TRN2", target_bir_lowering=False, debug=False,
                   num_devices=c.n_cores, num_swdge_queues=4)
    # I/O
    xs = nc.dram_tensor("xs", [c.SHARD, c.IN], FP32, kind="ExternalInput").ap()
    idx_all_d = nc.dram_tensor("idx_all", [P, idx_cols], mybir.dt.int16,
                               kind="ExternalInput").ap()
    sm_d = nc.dram_tensor("sm_all", [P, tot_chunks * P], HDT,
                          kind="ExternalInput").ap()
    w0cat_d = nc.dram_tensor("w0cat", [c.IN, c.F], BF, kind="ExternalInput").ap()
    b0rep_d = nc.dram_tensor("b0rep", [P, c.F], FP32, kind="ExternalInput").ap()
    w1cat_d = nc.dram_tensor("w1cat", [c.H, c.NETS * c.OUT], BF,
                             kind="ExternalInput").ap()
    b1rep_d = nc.dram_tensor("b1rep", [P, c.NETS * c.OUT], FP32,
                             kind="ExternalInput").ap()
    cwcat_d = nc.dram_tensor("cwcat", [c.L, c.H, c.F], BF, kind="ExternalInput").ap()
    identb_d = nc.dram_tensor("ident_b", [P, P], BF, kind="ExternalInput").ap()
    identf_d = nc.dram_tensor("ident_f", [P, P], FP32, kind="ExternalInput").ap()
    out_d = nc.dram_tensor("out", [c.SHARD, c.OUT], FP32, kind="ExternalOutput").ap()

    with tile.TileContext(nc) as tc:
        with tc.tile_pool(name="dram", bufs=1, space="DRAM") as dram, \
             tc.tile_pool(name="const", bufs=1) as cst:
            h_shard = [dram.tile([c.SHARD, ROW], HDT, name=f"h_shard{p}")
                       for p in range(2)]
            h_full_space = "Shared" if (c.n_cores > 4 and c.ag_shared) else "Local"
            h_full_all = [[dram.tile([c.N_pad, ROW], HDT,
                                     addr_space=h_full_space,
                                     name=f"h_full_r{r}_l{l}")
                           for l in range(c.L)] for r in range(c.reps)]
            x0s_hbm = dram.tile([c.SHARD, c.F], BF, name="x0s_hbm")
            h4_hbm = dram.tile([c.SHARD, c.F], BF, name="h4_hbm")

            # resident constants
            w0_sb = cst.tile([P, c.F], BF)
            b0_sb = cst.tile([P, c.F], FP32)
            w1_sb = cst.tile([P, c.NETS * c.OUT], BF)
            b1_sb = cst.tile([P, c.NETS * c.OUT], FP32)
            cw_sb = cst.tile([P, c.L * c.F], BF)
            idb_sb = cst.tile([P, P], BF)
            idf_sb = cst.tile([P, P], FP32)
            nc.sync.dma_start(out=w0_sb[:], in_=w0cat_d)
            nc.sync.dma_start(out=b0_sb[:], in_=b0rep_d)
            nc.sync.dma_start(out=w1_sb[:], in_=w1cat_d)
            nc.sync.dma_start(out=b1_sb[:], in_=b1rep_d)
            for l in range(c.L):
                nc.sync.dma_start(out=cw_sb[:, l * c.F:(l + 1) * c.F], in_=cwcat_d[l])
            nc.sync.dma_start(out=idb_sb[:], in_=identb_d)
            nc.sync.dma_start(out=idf_sb[:], in_=identf_d)

            def tile_is_real(t):
                return any(k * c.SHARD + t * P < c.N for k in range(c.n_cores))

            real_tiles = [t for t in range(c.TPC) if tile_is_real(t)]

            serial_reps = "serial" in ABL
            for rep in range(c.reps):
                # ---------------- Stage A: input layer ----------------
                with tc.tile_pool(name="sA", bufs=4) as sA, \
                     tc.tile_pool(name="pA", bufs=2, space="PSUM") as pA:
                    gate = None
                    if serial_reps and rep > 0:
                        od = sA.tile([P, c.OUT], FP32, tag="od", name=f"od{rep}")
                        nc.sync.dma_start(out=od[:], in_=out_d[0:P, :])
                        g0 = sA.tile([P, 1], FP32, tag="g0", name=f"g0{rep}")
                        nc.vector.tensor_reduce(
                            out=g0[:], in_=od[:], axis=mybir.AxisListType.X,
                            op=mybir.AluOpType.max)
                        gate = sA.tile([P, 1], FP32, tag="g1", name=f"g1{rep}")
                        nc.vector.tensor_scalar(
                            out=gate[:], in0=g0[:], scalar1=0.0, scalar2=1.0,
                            op0=mybir.AluOpType.mult, op1=mybir.AluOpType.add)
                    for t in real_tiles:
                        xt = sA.tile([P, c.IN], FP32, tag="xt")
                        nc.sync.dma_start(out=xt[:], in_=xs[t * P:(t + 1) * P, :])
                        if gate is not None:
                            xg = sA.tile([P, c.IN], FP32, tag="xg")
                            nc.vector.tensor_tensor(
                                out=xg[:], in0=xt[:],
                                in1=gate[:, 0:1].to_broadcast([P, c.IN]),
                                op=mybir.AluOpType.mult)
                            xt = xg
                        xT_ps = pA.tile([P, P], FP32, tag="xT")
                        nc.tensor.transpose(out=xT_ps[:], in_=xt[:], identity=idf_sb[:])
                        xT_sb = sA.tile([P, P], BF, tag="xTs")
                        nc.scalar.activation(out=xT_sb[:], in_=xT_ps[:],
                                             func=mybir.ActivationFunctionType.Copy)
                        h0_ps = pA.tile([P, c.F], FP32, tag="h0")
                        nc.tensor.matmul(out=h0_ps[:], lhsT=xT_sb[:], rhs=w0_sb[:],
                                         start=True, stop=True)
                        hb = sA.tile([P, c.F], FP32, tag="hb")
                        nc.vector.tensor_tensor(out=hb[:], in0=h0_ps[:], in1=b0_sb[:],
                                                op=mybir.AluOpType.add)
                        h0t = sA.tile([P, c.F], BF, tag="h0t")
                        nc.scalar.activation(out=h0t[:], in_=hb[:],
                                             func=mybir.ActivationFunctionType.Relu)
                        x0t = sA.tile([P, c.F], BF, tag="x0t")
                        nc.vector.tensor_scalar(out=x0t[:], in0=h0t[:],
                                                scalar1=ALPHA, scalar2=None,
                                                op0=mybir.AluOpType.mult)
                        if c.fp8:
                            hq = sA.tile([P, c.F], F8, tag="hq")
                            nc.scalar.activation(
                                out=hq[:], in_=hb[:],
                                func=mybir.ActivationFunctionType.Relu)
                        else:
                            hq = h0t
                        nc.sync.dma_start(
                            out=h_shard[0][t * P:(t + 1) * P, 0:c.F], in_=hq[:])
                        nc.sync.dma_start(
                            out=x0s_hbm[t * P:(t + 1) * P, :], in_=x0t[:])

                # AG0
                h_full = h_full_all[rep]
                if "noag" not in ABL:
                    nc.gpsimd.collective_compute(
                        "AllGather", mybir.AluOpType.bypass,
                        replica_groups=[list(range(c.n_cores))],
                        ins=[h_shard[0][:, :]], outs=[h_full[0][:, :]])
                else:
                    nc.sync.dma_start(
                        out=h_full[0][:c.SHARD, :], in_=h_shard[0][:, :])

                # ---------------- Stage B: GCN2 layers ----------------
                with tc.tile_pool(name="gpool", bufs=2) as gpool, \
                     tc.tile_pool(name="spool", bufs=2) as spool, \
                     tc.tile_pool(name="tpool", bufs=4) as tpool, \
                     tc.tile_pool(name="psA", bufs=6, space="PSUM") as ps_agg, \
                     tc.tile_pool(name="psT", bufs=2, space="PSUM") as ps_t:
                    for l in range(c.L):
                        pr, pw = l % 2, (l + 1) % 2
                        beta = c.betas[l]
                        gather_cnt = 0
                        for s, tiles in enumerate(sup_tiles):
                            calls = [cl for cl in call_list if cl[0] == s]
                            if not calls:
                                continue
                            g_tiles = {}
                            col_lo = min(cl[3] for cl in calls)
                            col_hi = max(cl[3] + cl[2] // 16 for cl in calls)
                            ix = spool.tile([P, col_hi - col_lo],
                                            mybir.dt.int16, tag="ix",
                                            name=f"ix_{l}_{s}")
                            nc.sync.dma_start(
                                out=ix[:], in_=idx_all_d[:, col_lo:col_hi])
                            for (s_, b, nidx, coloff, ch0) in calls:
                                nchk = nidx // P
                                g = gpool.tile([P, nchk, ROW], HDT, tag=f"g{b}")
                                if "nogather" in ABL:
                                    nc.vector.memset(g[:], 0)
                                if "nogather" not in ABL:
                                    nc.gpsimd.dma_gather(
                                        g[:],
                                        h_full[l][b * c.BUCKET:(b + 1) * c.BUCKET, :],
                                        ix[:, coloff - col_lo:
                                           coloff - col_lo + nidx // 16],
                                        nidx, nidx, ROW,
                                        single_packet=single_packet,
                                        queue_num=gather_cnt % 4)
                                gather_cnt += 1
                                g_tiles[b] = (g, ch0, nchk)
                            ch_lo = min(ch0 for (_, _, _, _, ch0) in calls)
                            ch_hi = max(ch0 + nidx // P
                                        for (_, _, nidx, _, ch0) in calls)
                            ng_all = ch_hi - ch_lo
                            smt = spool.tile([P, ng_all, P], HDT, tag="sm",
                                             name=f"sm_{l}_{s}")
                            nc.sync.dma_start(
                                out=smt[:],
                                in_=sm_d[:, ch_lo * P:ch_hi * P])
                            s_tiles = [(ch_lo, ch_hi, smt)]
                            tiles_with_chunks = sorted(
                                {chunk_tile[ci] for ci in range(ch_lo, ch_hi)})
                            agg = {t: ps_agg.tile([P, c.F], FP32, tag="agg",
                                                  name=f"agg{l}_{s}_{t}")
                                   for t in tiles_with_chunks}
                            for b, (g, ch0, nchk) in g_tiles.items():
                                for k in range(nchk):
                                    ci = ch0 + k
                                    t = chunk_tile[ci]
                                    g0s, g1s, sm = next(
                                        (a, b_, smt) for (a, b_, smt) in s_tiles
                                        if a <= ci < b_)
                                    if "nomm" not in ABL:
                                        nc.tensor.matmul(
                                            out=agg[t][:],
                                            lhsT=sm[:, ci - g0s, :],
                                            rhs=g[:, k, 0:c.F],
                                            start=start_flag[ci],
                                            stop=stop_flag[ci])
                                    elif start_flag[ci]:
                                        nc.tensor.matmul(
                                            out=agg[t][:],
                                            lhsT=sm[:, ci - g0s, :],
                                            rhs=g[:, k, 0:c.F],
                                            start=True, stop=True)
                            # dense per tile
                            for t in tiles:
                                if t not in agg:
                                    continue
                                x0t2 = tpool.tile([P, c.F], BF, tag="x0r")
                                nc.sync.dma_start(
                                    out=x0t2[:], in_=x0s_hbm[t * P:(t + 1) * P, :])
                                t_sb = tpool.tile([P, c.F], BF, tag="t")
                                nc.vector.tensor_tensor(
                                    out=t_sb[:], in0=agg[t][:], in1=x0t2[:],
                                    op=mybir.AluOpType.add)
                                tT_ps = ps_t.tile([P, c.F], BF, tag="tT")
                                for n in range(c.NETS):
                                    nc.tensor.transpose(
                                        out=tT_ps[:, n * c.H:(n + 1) * c.H],
                                        in_=t_sb[:, n * c.H:(n + 1) * c.H],
                                        identity=idb_sb[:])
                                tT_sb = tpool.tile([P, c.F], BF, tag="tTs")
                                nc.scalar.activation(
                                    out=tT_sb[:], in_=tT_ps[:],
                                    func=mybir.ActivationFunctionType.Copy)
                                # u = t @ cw accumulates straight onto the agg
                                # PSUM bank: afterwards agg = S.T@G + u, and
                                # s = t + u = agg + x0.
                                for n in range(c.NETS):
                                    nc.tensor.matmul(
                                        out=agg[t][:, n * c.H:(n + 1) * c.H],
                                        lhsT=tT_sb[:, n * c.H:(n + 1) * c.H],
                                        rhs=cw_sb[:, l * c.F + n * c.H:
                                                  l * c.F + (n + 1) * c.H],
                                        start=False, stop=True)
                                s_sb = tpool.tile([P, c.F], FP32, tag="s")
                                nc.vector.tensor_tensor(
                                    out=s_sb[:], in0=agg[t][:], in1=x0t2[:],
                                    op=mybir.AluOpType.add)
                                if l == c.L - 1:
                                    h_sb = tpool.tile([P, c.F], BF, tag="h")
                                    nc.scalar.activation(
                                        out=h_sb[:], in_=s_sb[:],
                                        func=mybir.ActivationFunctionType.Relu,
                                        scale=float(1.0 - beta))
                                    nc.sync.dma_start(
                                        out=h4_hbm[t * P:(t + 1) * P, :],
                                        in_=h_sb[:])
                                else:
                                    h_sb = tpool.tile([P, c.F], HDT, tag="h8")
                                    nc.scalar.activation(
                                        out=h_sb[:], in_=s_sb[:],
                                        func=mybir.ActivationFunctionType.Relu,
                                        scale=float(1.0 - beta))
                                    nc.sync.dma_start(
                                        out=h_shard[pw][t * P:(t + 1) * P, 0:c.F],
                                        in_=h_sb[:])
                        if l < c.L - 1:
                            if "noag" not in ABL:
                                nc.gpsimd.collective_compute(
                                    "AllGather", mybir.AluOpType.bypass,
                                    replica_groups=[list(range(c.n_cores))],
                                    ins=[h_shard[pw][:, :]],
                                    outs=[h_full[l + 1][:, :]])
                            else:
                                nc.sync.dma_start(
                                    out=h_full[l + 1][:c.SHARD, :],
                                    in_=h_shard[pw][:, :])

                # ---------------- Stage C: output layer ----------------
                NO = c.NETS * c.OUT
                with tc.tile_pool(name="sC", bufs=4) as sC, \
                     tc.tile_pool(name="pC", bufs=2, space="PSUM") as pC:
                    for t in real_tiles:
                        h4t = sC.tile([P, c.F], BF, tag="h4")
                        nc.sync.dma_start(out=h4t[:],
                                          in_=h4_hbm[t * P:(t + 1) * P, :])
                        hT_ps = pC.tile([P, c.F], BF, tag="hT")
                        for n in range(c.NETS):
                            nc.tensor.transpose(
                                out=hT_ps[:, n * c.H:(n + 1) * c.H],
                                in_=h4t[:, n * c.H:(n + 1) * c.H],
                                identity=idb_sb[:])
                        hT_sb = sC.tile([P, c.F], BF, tag="hTs")
                        nc.scalar.activation(out=hT_sb[:], in_=hT_ps[:],
                                             func=mybir.ActivationFunctionType.Copy)
                        o_ps = pC.tile([P, NO], FP32, tag="o")
                        for n in range(c.NETS):
                            nc.tensor.matmul(
                                out=o_ps[:, n * c.OUT:(n + 1) * c.OUT],
                                lhsT=hT_sb[:, n * c.H:(n + 1) * c.H],
                                rhs=w1_sb[:, n * c.OUT:(n + 1) * c.OUT],
                                start=True, stop=True)
                        ob = sC.tile([P, NO], FP32, tag="ob")
                        nc.vector.tensor_tensor(out=ob[:], in0=o_ps[:], in1=b1_sb[:],
                                                op=mybir.AluOpType.add)
                        logps = []
                        acc = sC.tile([P, c.OUT], FP32, tag="acc")
                        for n in range(c.NETS):
                            osl = ob[:, n * c.OUT:(n + 1) * c.OUT]
                            nmax = sC.tile([P, 1], FP32, tag=f"nmax{n}")
                            nc.vector.tensor_reduce(
                                out=nmax[:], in_=osl, axis=mybir.AxisListType.X,
                                op=mybir.AluOpType.max, negate=True)
                            e = sC.tile([P, c.OUT], FP32, tag=f"e{n}")
                            nc.scalar.activation(
                                out=e[:], in_=osl,
                                func=mybir.ActivationFunctionType.Exp,
                                bias=nmax[:, :1])
                            ssum = sC.tile([P, 1], FP32, tag=f"ss{n}")
                            nc.vector.tensor_reduce(
                                out=ssum[:], in_=e[:], axis=mybir.AxisListType.X,
                                op=mybir.AluOpType.add)
                            lsum = sC.tile([P, 1], FP32, tag=f"ls{n}")
                            nc.scalar.activation(
                                out=lsum[:], in_=ssum[:],
                                func=mybir.ActivationFunctionType.Ln)
                            lp = sC.tile([P, c.OUT], FP32, tag=f"lp{n}")
                            nc.vector.tensor_scalar(
                                out=lp[:], in0=osl,
                                scalar1=nmax[:, :1], scalar2=lsum[:, :1],
                                op0=mybir.AluOpType.add,
                                op1=mybir.AluOpType.subtract)
                            logps.append(lp)
                        nc.vector.tensor_tensor(out=acc[:], in0=logps[0][:],
                                                in1=logps[1][:],
                                                op=mybir.AluOpType.add)
                        nc.vector.tensor_tensor(out=acc[:], in0=acc[:],
                                                in1=logps[2][:],
                                                op=mybir.AluOpType.add)
                        outt = sC.tile([P, c.OUT], FP32, tag="outt")
                        nc.vector.tensor_scalar(
                            out=outt[:], in0=acc[:], scalar1=1.0 / 3.0,
                            scalar2=None, op0=mybir.AluOpType.mult)
                        nc.sync.dma_start(out=out_d[t * P:(t + 1) * P, :],
                                          in_=outt[:])
    nc.compile()
    return nc


# ----------------------------------------------------------------------------
# Public entry point
# ----------------------------------------------------------------------------
_CACHE = {}


def kernel(x, edge_index, edge_weight, lin0_w, lin0_b, lin1_w, lin1_b, conv_w):
    """GCN2Conv 3-net ensemble forward on 8 TRN2 NeuronCores.

    Node-sharded: 12544 rows/core (nodes padded to 100352). Edges are
    partitioned by destination; per layer the full [100352, 512] fp8
    activation table (3 nets fused, 512B-aligned rows) is AllGathered and
    source rows are fetched with dma_gather. Aggregation runs on the tensor
    engine via streamed one-hot matrices. Returns [100000, 112] float32
    log-probs.
    """
    import numpy as _np
    from concourse.bass_utils import run_bass_kernel_spmd

    cfg = Cfg(N=100000, IN=128, H=128, OUT=112, L=4, NETS=3,
              n_cores=8, n_buckets=4, t_sup=6, sg=16, reps=1)
    in_maps, schedule = host_prep(cfg, x, edge_index, edge_weight,
                                  lin0_w, lin0_b, lin1_w, lin1_b, conv_w)
    skey = (schedule["tot_chunks"], schedule["idx_cols"], cfg.fp8)
    if _CACHE.get("skey") != skey:
        _CACHE["nc"] = build_nc(cfg, schedule)
        _CACHE["skey"] = skey
    nc = _CACHE["nc"]
    res = run_bass_kernel_spmd(nc, in_maps, core_ids=list(range(cfg.n_cores)))
    out = _np.concatenate([res.results[k]["out"] for k in range(cfg.n_cores)],
                          axis=0)[:cfg.N]
    return _np.ascontiguousarray(out.astype(_np.float32))
